# revision 80
# baseline (speedup 1.0000x reference)
_C8_B64 = "Cx2VQ76vfEN9P3dDjox0QxvMu0LnF79CNUC7QlBZukJE87hC/za9QjWCuELPS71C+Vq9QvaEuEIBfLlC4yK7QnDWtkJq3rlCHuu3Qg+LvULzrrZCLE65QuJft0I5trtCxZe2QqCvukI3erdC+Ce3QmALu0JrzbRC0Je3QjPAvUI/DLdCEd26QrIDuUKw+bxCdxq2QuiKuUJ7rrlCmt+7QqcJu0KYFLdCsFi7Qnpst0JC2bVCUT+5QhUEuEJPTbpCcou3Qhdbt0J1wrdC04e1QjosvkJ+jrdCKLG5QmRYvUJ93LVCZj25Qia5tUIiu7tCmj21Qo1yt0IxSb1CCxi7Qq/WtkJpC7hC42q1QmrKukKCxLtCQoa3Qksxt0IpqLdC6v62QrrfuEL2mrpCW/K4QjCsvELZ5LdC3kO5QmirukIm+rlCVay4QrtLt0JqFLlC5Dy6QvQFt0LBXLlCyw64Qogru0IOJblCv8a5QrkyuELxGLpCjMm0Qm75u0LGrLpC/2S1QgXNuEKTXrhCaa+9QmWat0Kl8rpC6Da3QkqUukJj1rlCNAW3QmervkLWbbdCSxbAQkodtULEzb9CuJO5QnlBu0KK2rhCmni9QqSbt0IJqsBCRZO5QuGYvUJsZbpC3mi9QvbtvUKPbbxCn2C7QhqevkJ8Q75C6rm5QiySwEI="
"""CTC batch cost (keras ctc_batch_cost semantics) on 8 Trainium2 NeuronCores.

Strategy (pure data parallel, 32 examples per core):
  Linear-space CTC with an offline-tuned per-8-step scale schedule (C8) and a
  per-pair V2 tilt.

  Gather: y_pred loads in a contiguous layout (partition p holds t = 8p+tt,
  1536B DRAM runs per descriptor, so no sub-512B DMA penalty). The
  [t, c] -> [c, t] transpose runs on the tensor engine as 16 strided diagonal
  matmuls per example (8 tt families x 2 PSUM-bank halves); the C8 schedule
  rides the diagonal values for free. PSUM->SBUF bf16 cast copies round-robin
  across DVE and Act (the only engines that can read PSUM). A per-STATE
  one-hot matmul (E columns = 64 label lanes + 16 blank copies) emits the
  state series [state, t] per example; a per-example DRAM bounce (issued on
  the Pool SWDGE queue, keeping the busy Act/DVE SEQs free) plus h-major
  skew reads land everything in the wavefront layout
  ylab_sk[(chunk,example), (state-1+chunk)*256 + t'].

  Scans: T is split into 4 chunks of 256. Wavefront k packs 4 (state, chunk)
  cells (state s = k+1-c on partition group c) into ONE [128, 256]
  tensor_tensor_scan (d1 = ylab_sk[:, 256k:256(k+1)], initial = 0). The scan
  d0 is accumulated in PSUM by the tensor engine, which keeps the
  inter-wavefront critical path down to one 255-col diag matmul + one scan
  (~790ns/wavefront):
    d0s  = I @ wm_k        wm_k = mv2wave[:,k] * slot_{k-2} (DVE ts, 2-cycle
                           slack; its col 0 uses the Act-copied carry col)
    d0s[0]  += sh32 @ slot_{k-1}[TC]   (own-state chunk carry)
    d0s[0]  += (diag(c1)@sh32) @ slot_{k-2}[TC]  (s-1 chunk carry)
    d0s[1:] += diag(c1) @ slot_{k-1}[1:TC]       (on-path)
  The Ln activation table is prefired at startup so the final -ln(x)+K tail
  doesn't pay the 1.3us table load.
"""
import base64
import numpy as np
import ml_dtypes

B, T, C, L = 256, 1024, 96, 64
S = 2 * L + 1  # 129
BLANK = C - 1
EPS = 1e-7
NCORES = 8
BPC = B // NCORES  # 32 examples per core
NR = S - 1  # 128 grid rows (state s = r+1); state 0 handled separately
NCH = 4  # chunks
TC = T // NCH  # 256
NWAVE = NR + NCH - 1  # 131 wavefronts
GRP_LD = 2  # examples per load DMA
NE = 80  # gather rows: 64 label lanes + 16 blank copies (dedup of even states)

G = -2.25
V2 = np.float32(np.exp(2.0 * G))  # per-pair tilt factor

C8 = np.frombuffer(base64.b64decode(_C8_B64), dtype=np.float32).copy()  # [128]
C_SCHED = np.repeat(C8, 8)  # [T]
K_CORR = float(np.sum(np.log(C_SCHED.astype(np.float64))))
K_FIN = float(64.0 * np.log(np.float64(V2)) + K_CORR - 64.0 * np.log(2.0))

_PROGRAM = None


def _build_program(debug=False):
    import concourse.bacc as bacc
    import concourse.tile as tile
    import concourse.mybir as mybir

    f32 = mybir.dt.float32
    bf = mybir.dt.bfloat16
    ADD = mybir.AluOpType.add
    MULT = mybir.AluOpType.mult
    BYP = mybir.AluOpType.bypass

    nc = bacc.Bacc("TRN2", target_bir_lowering=False, debug=False, num_devices=NCORES)
    yp_d = nc.dram_tensor("y_pred", [BPC, T, C], bf, kind="ExternalInput")
    e_d = nc.dram_tensor("emat", [C, BPC * NE], bf, kind="ExternalInput")
    ds_d = nc.dram_tensor("dsched", [128, 8 * 128], bf, kind="ExternalInput")
    mw_d = nc.dram_tensor("mv2wave", [128, NWAVE], f32, kind="ExternalInput")
    sh_d = nc.dram_tensor("shift32", [128, 128], bf, kind="ExternalInput")
    # shift32 pre-scaled by c1 (two k-parity variants): shc1[par][q, p] =
    # c1_par[p] for q = p-32 — used to fold the a1 chunk-boundary term into
    # the d0 PSUM column 0, off the critical path.
    shc_d = nc.dram_tensor("shiftc1", [128, 2 * 128], bf, kind="ExternalInput")
    id_d = nc.dram_tensor("ident", [128, 128], bf, kind="ExternalInput")
    dc_d = nc.dram_tensor("diagc1", [128, 2 * 128], bf, kind="ExternalInput")
    out_d = nc.dram_tensor("out", [BPC, 1], f32, kind="ExternalOutput")
    if debug:
        ysk_d = nc.dram_tensor("ysk_dump", [128, NWAVE * TC], bf, kind="ExternalOutput")
        aw_d = nc.dram_tensor("aw_dump", [NWAVE, 128, TC + 1], bf, kind="ExternalOutput")
        NDBG = 6
        gbd_d = nc.dram_tensor("gbd_dump", [BPC, NE * T], bf, kind="ExternalOutput")
        gbs_d = nc.dram_tensor("gbs_dump", [NE, T], bf, kind="ExternalOutput")
        d0_d = nc.dram_tensor("d0_dump", [NDBG, 128, TC], bf, kind="ExternalOutput")
        wm_d = nc.dram_tensor("wm_dump", [NDBG, 128, TC], bf, kind="ExternalOutput")
        w4_d = nc.dram_tensor("w4_dump", [NDBG, 128, TC], bf, kind="ExternalOutput")
        sh_dump = nc.dram_tensor("sh_dump", [NDBG, 128, 1], f32, kind="ExternalOutput")

    with tile.TileContext(nc) as tc:
        with (
            tc.tile_pool(name="const", bufs=1) as const_pool,
            tc.tile_pool(name="tin", bufs=5) as tin_pool,
            tc.tile_pool(name="tcst", bufs=4) as tc_pool,
            tc.tile_pool(name="ypt", bufs=6) as ypt_pool,
            tc.tile_pool(name="gbs", bufs=5) as gbs_pool,
            tc.tile_pool(name="pstr", bufs=5, space="PSUM") as pstr_pool,
            tc.tile_pool(name="pst", bufs=3, space="PSUM") as pst_pool,
            tc.tile_pool(name="big", bufs=1) as big_pool,
            tc.tile_pool(name="scr", bufs=1, space="DRAM") as scr_pool,
            tc.tile_pool(name="w", bufs=3) as w_pool,
            tc.tile_pool(name="fin", bufs=1) as fin_pool,
        ):
            # ---- constants (only ds up front; the rest after the first
            # y_pred loads so they don't hold up the HWDGE at startup) ----
            ds_sb = const_pool.tile([128, 8 * 128], bf, tag="ds")
            nc.sync.dma_start(ds_sb[:], ds_d.ap())
            e_sb = const_pool.tile([C, BPC * NE], bf, tag="E")
            mw_sb = const_pool.tile([128, NWAVE], f32, tag="mw")
            sh_sb = const_pool.tile([128, 128], bf, tag="sh")
            shc_sb = const_pool.tile([128, 2 * 128], bf, tag="shc")
            id_sb = const_pool.tile([128, 128], bf, tag="id")
            dc_sb = const_pool.tile([128, 2 * 128], bf, tag="dc")

            # one extra TC of slack so the strided odd-state views fit
            ylab_sk = big_pool.tile([128, (NWAVE + 1) * TC], bf, tag="ysk")
            gbd = scr_pool.tile([BPC, NE * T], bf, tag="gbd")  # DRAM bounce
            ring = [
                big_pool.tile([128, TC + 1], bf, tag=f"aw{i}", name=f"aw{i}")
                for i in range(6)
            ]
            for r in ring:
                nc.gpsimd.memset(r[:], 0.0)
            # boundary 1.0 for state-1 cell (r=0,c=0): state0 at t=-1
            nc.gpsimd.memset(ring[5][0:32, 0:1], 1.0)
            # prefire the Ln activation-table load (1.3us) during startup:
            # otherwise it serializes into the final-loss tail
            warm = fin_pool.tile([128, 1], f32, tag="warm")
            nc.gpsimd.memset(warm[:], 1.0)
            nc.scalar.activation(
                warm[0:1, :], warm[0:1, :], mybir.ActivationFunctionType.Ln,
                scale=1.0,
            )
            # zero unwritten-but-read ylab_sk strips (NaN safety for idle cells)
            for c in range(1, NCH):
                nc.gpsimd.memset(ylab_sk[32 * c : 32 * (c + 1), 0 : c * TC], 0.0)
            for c in range(0, NCH - 1):
                nc.gpsimd.memset(
                    ylab_sk[32 * c : 32 * (c + 1), (NR + c) * TC : NWAVE * TC], 0.0
                )

            # ---- gather ----
            # tin partition p holds t = 8*p + tt (tt in 0..7): the source run
            # per (partition, example) is 8*96*2 = 1536B contiguous DRAM, so
            # the load avoids the sub-512B DMA penalty. The transpose is 8
            # diagonal matmuls (one per tt family), each writing 128 stride-8
            # psum cols; the C8 schedule rides the diagonal for free.
            ypa = yp_d.ap()
            prev_e = []
            # PSUM->SBUF cast copies balanced across DVE/Act (Pool cannot
            # read PSUM), weighted by engine speed (DVE 658ns, Act 612ns).
            _rr_pat = ["a", "v"]
            _rr = [0]

            def _copy_half(dst, src):
                e = _rr_pat[_rr[0] % len(_rr_pat)]
                _rr[0] += 1
                if e == "v":
                    nc.vector.tensor_copy(dst, src)
                elif e == "a":
                    nc.scalar.copy(dst, src)
                else:
                    nc.gpsimd.tensor_copy(dst, src)

            def _emit_e(pe):
                bb, yptb = pe
                gbs = gbs_pool.tile([NE, T], bf, tag="gbs")
                for half in range(2):
                    gps = pst_pool.tile(
                        [NE, T // 2], f32, tag="pst", name=f"gps{bb}_{half}"
                    )
                    nc.tensor.matmul(
                        gps[:],
                        e_sb[:, bb * NE : (bb + 1) * NE],
                        yptb[:, half * 512 : (half + 1) * 512],
                        start=True,
                        stop=True,
                    )
                    _copy_half(gbs[:, half * 512 : (half + 1) * 512], gps[:])
                if debug and bb == 0:
                    nc.sync.dma_start(gbs_d.ap()[:, :], gbs[:])
                # dump [row, t] block to DRAM (per example). Issued from the
                # otherwise-idle Pool engine (SWDGE): the HWDGE issue path
                # costs ~1.3us of the issuing SEQ per example.
                nc.gpsimd.dma_start(gbd[bb : bb + 1, :], gbs[:])

            for g in range(BPC // GRP_LD):
                tin = tin_pool.tile([128, GRP_LD * 8 * C], bf, tag="tin")
                nc.sync.dma_start(
                    tin[:],
                    ypa[g * GRP_LD : (g + 1) * GRP_LD].rearrange(
                        "e (p tt) c -> p e (tt c)", p=128, tt=8
                    ),
                )
                if g == 1:
                    # constants issued on the SP queue BEHIND the first two
                    # y_pred loads: from an idle queue (e.g. Act) they issue
                    # at t=0 and their ~3.3us of transfers (e_sb alone is
                    # 1.4us) delay the first transposes on the shared HWDGE
                    nc.sync.dma_start(e_sb[:], e_d.ap())
                    nc.sync.dma_start(mw_sb[:], mw_d.ap())
                    nc.sync.dma_start(sh_sb[:], sh_d.ap())
                    nc.sync.dma_start(shc_sb[:], shc_d.ap())
                    nc.sync.dma_start(id_sb[:], id_d.ap())
                    nc.sync.dma_start(dc_sb[:], dc_d.ap())
                for bl in range(GRP_LD):
                    b = g * GRP_LD + bl
                    ypt = ypt_pool.tile([C, T], bf, tag="ypt")
                    for half in range(2):
                        # one PSUM bank per half: t = 512*half + 8*jl + tt,
                        # source partition 64*half + jl
                        pstr = pstr_pool.tile(
                            [C, T // 2], f32, tag="pstr", name=f"yps{b}_{half}"
                        )
                        pview = pstr[:].rearrange("p (j s) -> p j s", s=8)
                        for tt in range(8):
                            nc.tensor.matmul(
                                pview[:, :, tt : tt + 1],
                                tin[
                                    64 * half : 64 * half + 64,
                                    (bl * 8 + tt) * C : (bl * 8 + tt + 1) * C,
                                ],
                                ds_sb[
                                    64 * half : 64 * half + 64,
                                    tt * 128 + 64 * half : tt * 128 + 64 * half + 64,
                                ],
                                start=True,
                                stop=True,
                            )
                        _copy_half(
                            ypt[:, half * 512 : (half + 1) * 512], pstr[:]
                        )
                    # E-matmuls TWO examples behind, so the PE queue never
                    # head-of-line blocks on a fresh example's copies.
                    if len(prev_e) >= 4:
                        _emit_e(prev_e.pop(0))
                    prev_e.append((b, ypt))
            for pe in prev_e:
                _emit_e(pe)
            prev_e = []

            # ---- build skewed wavefront layout from the DRAM bounce ----
            # ylab_sk[32c+b, (r+c)*TC + j] = series of state r+1 chunk c:
            # odd states (r even) from label-lane rows, even states (r odd)
            # from the 16 blank-copy rows. h-major order: wavefront k only
            # needs h <= k/32, so early scans start after the first DMAs.
            gba = gbd[:].rearrange("b (r t) -> b r t", t=T)
            for h in range(4):
                for cc in range(NCH):
                    base = 32 * h + cc
                    # label lanes: states r = 32h + 2i -> lane 16h + i
                    src = gba[:, 16 * h : 16 * (h + 1), cc * TC : (cc + 1) * TC]
                    dste = ylab_sk[
                        32 * cc : 32 * (cc + 1), base * TC : (base + 32) * TC
                    ].rearrange("b (i x) -> b i x", x=2 * TC)[:, :, 0:TC]
                    # blank: states r = 32h + 2i + 1 -> copy rows 64..79
                    srcb = gba[:, 64:80, cc * TC : (cc + 1) * TC]
                    dsto = ylab_sk[
                        32 * cc : 32 * (cc + 1), (base + 1) * TC : (base + 33) * TC
                    ].rearrange("b (i x) -> b i x", x=2 * TC)[:, :, 0:TC]
                    if h == 0 and cc == 0:
                        # blank block first: the state-0 cumprod (and thus
                        # the whole scan chain) depends on it
                        nc.sync.dma_start(dsto, srcb)
                        nc.sync.dma_start(dste, src)
                    else:
                        nc.sync.dma_start(dste, src)
                        nc.sync.dma_start(dsto, srcb)

            if debug:
                nc.sync.dma_start(ysk_d.ap()[:, :], ylab_sk[:])
                nc.sync.dma_start(gbd_d.ap()[:, :], gbd[:, :])

            # ---- scan phase ----
            def d1_view(k):
                return ylab_sk[:, k * TC : (k + 1) * TC]

            for k in range(NWAVE):
                if k == 0:
                    # state-0 (blank lane) cumprod, chunk 0 only: the series
                    # underflows bf16 to zero well before chunk 1, and the
                    # zero-memset ring tiles already supply zeros for chunks
                    # 1-3. Written into ring[5] group-0 rows so wavefront 0's
                    # packed d0 read sees it as "aw_{-1}".
                    slot = ring[5]
                    pbv = ylab_sk[0:32, TC : 2 * TC]  # pblank chunk 0 (r=1)
                    nc.vector.tensor_tensor_scan(
                        slot[0:32, 1 : TC + 1], pbv, pbv, 1.0, op0=MULT, op1=BYP
                    )

                slot_out = ring[k % 6]
                slot_1 = ring[(k - 1) % 6]
                slot_2 = ring[(k - 2) % 6]
                # d0t tile: DVE pre-writes the full skip term wm =
                # mv2wave[:,k] * STORED(s-2) (incl. boundary col 0, whose
                # slot_2 col 0 is the 2-cycle-old carry copy). Off chain (2
                # cycles of slack).
                d0t = w_pool.tile([128, TC], bf, tag="d0t", name=f"d0t{k}")
                nc.vector.tensor_scalar(
                    d0t[:], slot_2[:, 0:TC], mw_sb[:, k : k + 1], None, op0=MULT
                )
                # d0 accumulates in PSUM on the PE so the only critical-path
                # step between scans is one 255-col diag matmul:
                #   d0s  = I @ wm                       (early: wm ready)
                #   d0s[0]   += sh32 @ slot_1[TC]       (own-carry; initial
                #   d0s[0]   += shc1 @ slot_2[TC]        and a1-carry folded
                #                                        into col 0)
                #   d0s[1:]  += diag(c1) @ slot_1[1:TC] (on-path, after
                #                                        scan_{k-1})
                # The scan then runs with initial=0.
                d0s = pstr_pool.tile([128, TC], f32, tag="pstr", name=f"d0s{k}")
                nc.tensor.matmul(d0s[:], id_sb[:], d0t[:], start=True, stop=False)
                shp = pst_pool.tile([128, 8], f32, tag="pst", name=f"shp{k}")
                nc.tensor.matmul(
                    shp[:, 1:2], sh_sb[:], slot_1[:, TC : TC + 1],
                    start=True, stop=True,
                )
                if k == 0:
                    # ring[4] col 0 carries the state-0 t=-1 boundary (1.0):
                    # cover col 0 with the diag matmul directly.
                    nc.tensor.matmul(
                        d0s[:, 0:1], sh_sb[:], slot_1[:, TC : TC + 1],
                        start=False, stop=False,
                    )
                    nc.tensor.matmul(
                        d0s[:, 0:TC], dc_sb[:, 0:128], slot_1[:, 0:TC],
                        start=False, stop=True,
                    )
                else:
                    nc.tensor.matmul(
                        d0s[:, 0:1], sh_sb[:], slot_1[:, TC : TC + 1],
                        start=False, stop=False,
                    )
                    nc.tensor.matmul(
                        d0s[:, 0:1],
                        shc_sb[:, (k % 2) * 128 : (k % 2) * 128 + 128],
                        slot_2[:, TC : TC + 1],
                        start=False, stop=False,
                    )
                    nc.tensor.matmul(
                        d0s[:, 1:TC],
                        dc_sb[:, (k % 2) * 128 : (k % 2) * 128 + 128],
                        slot_1[:, 1:TC],
                        start=False, stop=True,
                    )
                nc.vector.tensor_tensor_scan(
                    slot_out[:, 1 : TC + 1],
                    d0s[:, 0:TC],
                    d1_view(k),
                    0.0,
                    op0=ADD,
                    op1=MULT,
                )
                # boundary col 0 of THIS slot (consumed by the wm op of
                # wavefront k+2, which has ~2 cycles of slack). Emitted after
                # the scan so the scheduler doesn't gate the scan behind it.
                nc.scalar.copy(ring[k % 6][:, 0:1], shp[:, 1:2])
                if debug:
                    nc.sync.dma_start(aw_d.ap()[k], slot_out[:])
                    if k < 6:
                        nc.sync.dma_start(d0_d.ap()[k], d0t[:])
                        nc.sync.dma_start(wm_d.ap()[k], wm[:])
                        nc.sync.dma_start(w4_d.ap()[k], w4[:])
                        shcp = w_pool.tile([128, 1], f32, tag="shcp", name=f"shcp{k}")
                        nc.scalar.copy(shcp[:], shp[:, 0:1])
                        nc.sync.dma_start(sh_dump.ap()[k], shcp[:])

            # ---- final ----
            # STORED[127] from wavefront 129 (ring[4]), STORED[128] from 130
            # (ring[0]); both group 3, last col.
            xa = ring[129 % 6][96:128, TC : TC + 1]
            xb = ring[130 % 6][96:128, TC : TC + 1]
            xt = fin_pool.tile([128, 1], f32, tag="x")
            nc.vector.tensor_tensor(xt[96:128, :], xa, xb, op=ADD)
            lnx = fin_pool.tile([128, 1], f32, tag="lnx")
            nc.scalar.activation(
                lnx[96:128, :],
                xt[96:128, :],
                mybir.ActivationFunctionType.Ln,
                scale=float(2.0**-64),
            )
            res = fin_pool.tile([128, 1], f32, tag="res")
            nc.vector.tensor_scalar(res[96:128, :], lnx[96:128, :], -1.0, K_FIN, MULT, ADD)
            nc.sync.dma_start(out_d.ap()[:, :], res[96:128, :])

    nc.compile()
    return nc


def _host_inputs(y_true, y_pred):
    """Per-core input maps."""
    bf16 = ml_dtypes.bfloat16
    # shared constants
    # family tt: moving col j -> t = 8j + tt, source partition j
    dsched = np.zeros((128, 8 * 128), dtype=bf16)
    for tt in range(8):
        for j in range(128):
            dsched[j, tt * 128 + j] = bf16(C_SCHED[8 * j + tt])
    v2tab = np.zeros((128, 3), dtype=np.float32)
    for c in range(4):
        for j in range(2):
            v2tab[32 * c : 32 * (c + 1), j] = V2 if (c % 2) == j else 1.0
    v2tab[:, 2] = EPS
    shift32 = np.zeros((128, 128), dtype=bf16)
    for p in range(96):
        shift32[p, p + 32] = bf16(1.0)
    # shift32 pre-scaled by c1 (k-parity variants j=0,1): out[p] = c1_j[p] *
    # src[p-32] -> stationary[q, j*128 + p] = v2tab[p, j] at p = q+32.
    shiftc1 = np.zeros((128, 2 * 128), dtype=bf16)
    for q in range(96):
        for j in range(2):
            shiftc1[q, j * 128 + q + 32] = bf16(v2tab[q + 32, j])
    ident = np.zeros((128, 128), dtype=bf16)
    for q in range(128):
        ident[q, q] = bf16(1.0)
    # diag(c1) per k-parity: diagc1[q, j*128 + q] = v2tab[q, j]
    diagc1 = np.zeros((128, 2 * 128), dtype=bf16)
    for q in range(128):
        for j in range(2):
            diagc1[q, j * 128 + q] = bf16(v2tab[q, j])

    in_maps = []
    for i in range(NCORES):
        sl = slice(i * BPC, (i + 1) * BPC)
        lab = np.asarray(y_true[sl], dtype=np.int64)  # [32, 64]
        # rows 0..63 = label lanes, rows 64..79 = blank copies
        emat = np.zeros((C, BPC * NE), dtype=bf16)
        for b in range(BPC):
            for l in range(L):
                emat[lab[b, l], b * NE + l] = bf16(1.0)
            emat[BLANK, b * NE + 64 : b * NE + 80] = bf16(1.0)
        mv2wave = np.zeros((128, NWAVE), dtype=np.float32)
        for c in range(4):
            for k in range(NWAVE):
                s = k + 1 - c
                if 3 <= s <= NR and s % 2 == 1:
                    l = (s - 1) // 2
                    mv2wave[32 * c : 32 * (c + 1), k] = (
                        lab[:, l] != lab[:, l - 1]
                    ).astype(np.float32) * V2
        in_maps.append(
            {
                "y_pred": np.ascontiguousarray(
                    (np.asarray(y_pred[sl], np.float32) + np.float32(EPS)).astype(bf16)
                ),
                "emat": emat,
                "dsched": dsched,
                "mv2wave": mv2wave,
                "shift32": shift32,
                "shiftc1": shiftc1,
                "ident": ident,
                "diagc1": diagc1,
            }
        )
    return in_maps


def kernel(y_true, y_pred):
    global _PROGRAM
    from concourse.bass_utils import run_bass_kernel_spmd

    y_true = np.asarray(y_true)
    y_pred = np.asarray(y_pred, dtype=np.float32)
    if _PROGRAM is None:
        _PROGRAM = _build_program()
    in_maps = _host_inputs(y_true, y_pred)
    r = run_bass_kernel_spmd(_PROGRAM, in_maps, list(range(NCORES)))
    out = np.concatenate([r.results[i]["out"] for i in range(NCORES)], axis=0)
    return out.astype(np.float32)



# revision 81
# speedup vs baseline: 1.0001x; 1.0001x over previous
_C8_B64 = "Cx2VQ76vfEN9P3dDjox0QxvMu0LnF79CNUC7QlBZukJE87hC/za9QjWCuELPS71C+Vq9QvaEuEIBfLlC4yK7QnDWtkJq3rlCHuu3Qg+LvULzrrZCLE65QuJft0I5trtCxZe2QqCvukI3erdC+Ce3QmALu0JrzbRC0Je3QjPAvUI/DLdCEd26QrIDuUKw+bxCdxq2QuiKuUJ7rrlCmt+7QqcJu0KYFLdCsFi7Qnpst0JC2bVCUT+5QhUEuEJPTbpCcou3Qhdbt0J1wrdC04e1QjosvkJ+jrdCKLG5QmRYvUJ93LVCZj25Qia5tUIiu7tCmj21Qo1yt0IxSb1CCxi7Qq/WtkJpC7hC42q1QmrKukKCxLtCQoa3Qksxt0IpqLdC6v62QrrfuEL2mrpCW/K4QjCsvELZ5LdC3kO5QmirukIm+rlCVay4QrtLt0JqFLlC5Dy6QvQFt0LBXLlCyw64Qogru0IOJblCv8a5QrkyuELxGLpCjMm0Qm75u0LGrLpC/2S1QgXNuEKTXrhCaa+9QmWat0Kl8rpC6Da3QkqUukJj1rlCNAW3QmervkLWbbdCSxbAQkodtULEzb9CuJO5QnlBu0KK2rhCmni9QqSbt0IJqsBCRZO5QuGYvUJsZbpC3mi9QvbtvUKPbbxCn2C7QhqevkJ8Q75C6rm5QiySwEI="
"""CTC batch cost (keras ctc_batch_cost semantics) on 8 Trainium2 NeuronCores.

Strategy (pure data parallel, 32 examples per core):
  Linear-space CTC with an offline-tuned per-8-step scale schedule (C8) and a
  per-pair V2 tilt.

  Gather: y_pred loads in a contiguous layout (partition p holds t = 8p+tt,
  1536B DRAM runs per descriptor, so no sub-512B DMA penalty). The
  [t, c] -> [c, t] transpose runs on the tensor engine as 16 strided diagonal
  matmuls per example (8 tt families x 2 PSUM-bank halves); the C8 schedule
  rides the diagonal values for free. PSUM->SBUF bf16 cast copies round-robin
  across DVE and Act (the only engines that can read PSUM). A per-STATE
  one-hot matmul (E columns = 64 label lanes + 16 blank copies) emits the
  state series [state, t] per example; a per-example DRAM bounce (issued on
  the Pool SWDGE queue, keeping the busy Act/DVE SEQs free) plus h-major
  skew reads land everything in the wavefront layout
  ylab_sk[(chunk,example), (state-1+chunk)*256 + t'].

  Scans: T is split into 4 chunks of 256. Wavefront k packs 4 (state, chunk)
  cells (state s = k+1-c on partition group c) into ONE [128, 256]
  tensor_tensor_scan (d1 = ylab_sk[:, 256k:256(k+1)], initial = 0). The scan
  d0 is accumulated in PSUM by the tensor engine, which keeps the
  inter-wavefront critical path down to one 255-col diag matmul + one scan
  (~790ns/wavefront):
    d0s  = I @ wm_k        wm_k = mv2wave[:,k] * slot_{k-2} (DVE ts, 2-cycle
                           slack; its col 0 uses the Act-copied carry col)
    d0s[0]  += sh32 @ slot_{k-1}[TC]   (own-state chunk carry)
    d0s[0]  += (diag(c1)@sh32) @ slot_{k-2}[TC]  (s-1 chunk carry)
    d0s[1:] += diag(c1) @ slot_{k-1}[1:TC]       (on-path)
  The Ln activation table is prefired at startup so the final -ln(x)+K tail
  doesn't pay the 1.3us table load.
"""
import base64
import numpy as np
import ml_dtypes

B, T, C, L = 256, 1024, 96, 64
S = 2 * L + 1  # 129
BLANK = C - 1
EPS = 1e-7
NCORES = 8
BPC = B // NCORES  # 32 examples per core
NR = S - 1  # 128 grid rows (state s = r+1); state 0 handled separately
NCH = 4  # chunks
TC = T // NCH  # 256
NWAVE = NR + NCH - 1  # 131 wavefronts
GRP_LD = 2  # examples per load DMA
NE = 80  # gather rows: 64 label lanes + 16 blank copies (dedup of even states)

G = -2.25
V2 = np.float32(np.exp(2.0 * G))  # per-pair tilt factor

C8 = np.frombuffer(base64.b64decode(_C8_B64), dtype=np.float32).copy()  # [128]
C_SCHED = np.repeat(C8, 8)  # [T]
K_CORR = float(np.sum(np.log(C_SCHED.astype(np.float64))))
K_FIN = float(64.0 * np.log(np.float64(V2)) + K_CORR - 64.0 * np.log(2.0))

_PROGRAM = None


def _build_program(debug=False):
    import concourse.bacc as bacc
    import concourse.tile as tile
    import concourse.mybir as mybir

    f32 = mybir.dt.float32
    bf = mybir.dt.bfloat16
    ADD = mybir.AluOpType.add
    MULT = mybir.AluOpType.mult
    BYP = mybir.AluOpType.bypass

    nc = bacc.Bacc("TRN2", target_bir_lowering=False, debug=False, num_devices=NCORES)
    yp_d = nc.dram_tensor("y_pred", [BPC, T, C], bf, kind="ExternalInput")
    e_d = nc.dram_tensor("emat", [C, BPC * NE], bf, kind="ExternalInput")
    ds_d = nc.dram_tensor("dsched", [128, 8 * 128], bf, kind="ExternalInput")
    mw_d = nc.dram_tensor("mv2wave", [128, NWAVE], f32, kind="ExternalInput")
    sh_d = nc.dram_tensor("shift32", [128, 128], bf, kind="ExternalInput")
    # shift32 pre-scaled by c1 (two k-parity variants): shc1[par][q, p] =
    # c1_par[p] for q = p-32 — used to fold the a1 chunk-boundary term into
    # the d0 PSUM column 0, off the critical path.
    shc_d = nc.dram_tensor("shiftc1", [128, 2 * 128], bf, kind="ExternalInput")
    id_d = nc.dram_tensor("ident", [128, 128], bf, kind="ExternalInput")
    dc_d = nc.dram_tensor("diagc1", [128, 2 * 128], bf, kind="ExternalInput")
    out_d = nc.dram_tensor("out", [BPC, 1], f32, kind="ExternalOutput")
    if debug:
        ysk_d = nc.dram_tensor("ysk_dump", [128, NWAVE * TC], bf, kind="ExternalOutput")
        aw_d = nc.dram_tensor("aw_dump", [NWAVE, 128, TC + 1], bf, kind="ExternalOutput")
        NDBG = 6
        gbd_d = nc.dram_tensor("gbd_dump", [BPC, NE * T], bf, kind="ExternalOutput")
        gbs_d = nc.dram_tensor("gbs_dump", [NE, T], bf, kind="ExternalOutput")
        d0_d = nc.dram_tensor("d0_dump", [NDBG, 128, TC], bf, kind="ExternalOutput")
        wm_d = nc.dram_tensor("wm_dump", [NDBG, 128, TC], bf, kind="ExternalOutput")
        w4_d = nc.dram_tensor("w4_dump", [NDBG, 128, TC], bf, kind="ExternalOutput")
        sh_dump = nc.dram_tensor("sh_dump", [NDBG, 128, 1], f32, kind="ExternalOutput")

    with tile.TileContext(nc) as tc:
        with (
            tc.tile_pool(name="const", bufs=1) as const_pool,
            tc.tile_pool(name="tin", bufs=5) as tin_pool,
            tc.tile_pool(name="tcst", bufs=4) as tc_pool,
            tc.tile_pool(name="ypt", bufs=6) as ypt_pool,
            tc.tile_pool(name="gbs", bufs=5) as gbs_pool,
            tc.tile_pool(name="pstr", bufs=5, space="PSUM") as pstr_pool,
            tc.tile_pool(name="pst", bufs=3, space="PSUM") as pst_pool,
            tc.tile_pool(name="big", bufs=1) as big_pool,
            tc.tile_pool(name="scr", bufs=1, space="DRAM") as scr_pool,
            tc.tile_pool(name="w", bufs=3) as w_pool,
            tc.tile_pool(name="fin", bufs=1) as fin_pool,
        ):
            # ---- constants (only ds up front; the rest after the first
            # y_pred loads so they don't hold up the HWDGE at startup) ----
            ds_sb = const_pool.tile([128, 8 * 128], bf, tag="ds")
            nc.sync.dma_start(ds_sb[:], ds_d.ap())
            e_sb = const_pool.tile([C, BPC * NE], bf, tag="E")
            mw_sb = const_pool.tile([128, NWAVE], f32, tag="mw")
            sh_sb = const_pool.tile([128, 128], bf, tag="sh")
            shc_sb = const_pool.tile([128, 2 * 128], bf, tag="shc")
            id_sb = const_pool.tile([128, 128], bf, tag="id")
            dc_sb = const_pool.tile([128, 2 * 128], bf, tag="dc")

            # one extra TC of slack so the strided odd-state views fit
            ylab_sk = big_pool.tile([128, (NWAVE + 1) * TC], bf, tag="ysk")
            gbd = scr_pool.tile([BPC, NE * T], bf, tag="gbd")  # DRAM bounce
            ring = [
                big_pool.tile([128, TC + 1], bf, tag=f"aw{i}", name=f"aw{i}")
                for i in range(6)
            ]
            for r in ring:
                nc.gpsimd.memset(r[:], 0.0)
            # boundary 1.0 for state-1 cell (r=0,c=0): state0 at t=-1
            nc.gpsimd.memset(ring[5][0:32, 0:1], 1.0)
            # prefire the Ln activation-table load (1.3us) during startup:
            # otherwise it serializes into the final-loss tail
            warm = fin_pool.tile([128, 1], f32, tag="warm")
            nc.gpsimd.memset(warm[:], 1.0)
            nc.scalar.activation(
                warm[0:1, :], warm[0:1, :], mybir.ActivationFunctionType.Ln,
                scale=1.0,
            )
            # zero unwritten-but-read ylab_sk strips (NaN safety for idle cells)
            for c in range(1, NCH):
                nc.gpsimd.memset(ylab_sk[32 * c : 32 * (c + 1), 0 : c * TC], 0.0)
            for c in range(0, NCH - 1):
                nc.gpsimd.memset(
                    ylab_sk[32 * c : 32 * (c + 1), (NR + c) * TC : NWAVE * TC], 0.0
                )

            # ---- gather ----
            # tin partition p holds t = 8*p + tt (tt in 0..7): the source run
            # per (partition, example) is 8*96*2 = 1536B contiguous DRAM, so
            # the load avoids the sub-512B DMA penalty. The transpose is 8
            # diagonal matmuls (one per tt family), each writing 128 stride-8
            # psum cols; the C8 schedule rides the diagonal for free.
            ypa = yp_d.ap()
            prev_e = []
            # PSUM->SBUF cast copies balanced across DVE/Act (Pool cannot
            # read PSUM), weighted by engine speed (DVE 658ns, Act 612ns).
            _rr_pat = ["a", "v"]
            _rr = [0]

            def _copy_half(dst, src):
                e = _rr_pat[_rr[0] % len(_rr_pat)]
                _rr[0] += 1
                if e == "v":
                    nc.vector.tensor_copy(dst, src)
                elif e == "a":
                    nc.scalar.copy(dst, src)
                else:
                    nc.gpsimd.tensor_copy(dst, src)

            def _emit_e(pe):
                bb, yptb = pe
                gbs = gbs_pool.tile([NE, T], bf, tag="gbs")
                for half in range(2):
                    gps = pst_pool.tile(
                        [NE, T // 2], f32, tag="pst", name=f"gps{bb}_{half}"
                    )
                    nc.tensor.matmul(
                        gps[:],
                        e_sb[:, bb * NE : (bb + 1) * NE],
                        yptb[:, half * 512 : (half + 1) * 512],
                        start=True,
                        stop=True,
                    )
                    _copy_half(gbs[:, half * 512 : (half + 1) * 512], gps[:])
                if debug and bb == 0:
                    nc.sync.dma_start(gbs_d.ap()[:, :], gbs[:])
                # dump [row, t] block to DRAM (per example). Issued from the
                # otherwise-idle Pool engine (SWDGE): the HWDGE issue path
                # costs ~1.3us of the issuing SEQ per example.
                nc.gpsimd.dma_start(gbd[bb : bb + 1, :], gbs[:])

            for g in range(BPC // GRP_LD):
                tin = tin_pool.tile([128, GRP_LD * 8 * C], bf, tag="tin")
                nc.sync.dma_start(
                    tin[:],
                    ypa[g * GRP_LD : (g + 1) * GRP_LD].rearrange(
                        "e (p tt) c -> p e (tt c)", p=128, tt=8
                    ),
                )
                if g == 1:
                    # constants issued on the SP queue BEHIND the first two
                    # y_pred loads: from an idle queue (e.g. Act) they issue
                    # at t=0 and their ~3.3us of transfers (e_sb alone is
                    # 1.4us) delay the first transposes on the shared HWDGE
                    nc.sync.dma_start(e_sb[:], e_d.ap())
                    nc.sync.dma_start(mw_sb[:], mw_d.ap())
                    nc.sync.dma_start(sh_sb[:], sh_d.ap())
                    nc.sync.dma_start(shc_sb[:], shc_d.ap())
                    nc.sync.dma_start(id_sb[:], id_d.ap())
                    nc.sync.dma_start(dc_sb[:], dc_d.ap())
                for bl in range(GRP_LD):
                    b = g * GRP_LD + bl
                    ypt = ypt_pool.tile([C, T], bf, tag="ypt")
                    for half in range(2):
                        # one PSUM bank per half: t = 512*half + 8*jl + tt,
                        # source partition 64*half + jl
                        pstr = pstr_pool.tile(
                            [C, T // 2], f32, tag="pstr", name=f"yps{b}_{half}"
                        )
                        pview = pstr[:].rearrange("p (j s) -> p j s", s=8)
                        for tt in range(8):
                            nc.tensor.matmul(
                                pview[:, :, tt : tt + 1],
                                tin[
                                    64 * half : 64 * half + 64,
                                    (bl * 8 + tt) * C : (bl * 8 + tt + 1) * C,
                                ],
                                ds_sb[
                                    64 * half : 64 * half + 64,
                                    tt * 128 + 64 * half : tt * 128 + 64 * half + 64,
                                ],
                                start=True,
                                stop=True,
                            )
                        _copy_half(
                            ypt[:, half * 512 : (half + 1) * 512], pstr[:]
                        )
                    # E-matmuls TWO examples behind, so the PE queue never
                    # head-of-line blocks on a fresh example's copies.
                    if len(prev_e) >= 3:
                        _emit_e(prev_e.pop(0))
                    prev_e.append((b, ypt))
            for pe in prev_e:
                _emit_e(pe)
            prev_e = []

            # ---- build skewed wavefront layout from the DRAM bounce ----
            # ylab_sk[32c+b, (r+c)*TC + j] = series of state r+1 chunk c:
            # odd states (r even) from label-lane rows, even states (r odd)
            # from the 16 blank-copy rows. h-major order: wavefront k only
            # needs h <= k/32, so early scans start after the first DMAs.
            gba = gbd[:].rearrange("b (r t) -> b r t", t=T)
            for h in range(4):
                for cc in range(NCH):
                    base = 32 * h + cc
                    # label lanes: states r = 32h + 2i -> lane 16h + i
                    src = gba[:, 16 * h : 16 * (h + 1), cc * TC : (cc + 1) * TC]
                    dste = ylab_sk[
                        32 * cc : 32 * (cc + 1), base * TC : (base + 32) * TC
                    ].rearrange("b (i x) -> b i x", x=2 * TC)[:, :, 0:TC]
                    # blank: states r = 32h + 2i + 1 -> copy rows 64..79
                    srcb = gba[:, 64:80, cc * TC : (cc + 1) * TC]
                    dsto = ylab_sk[
                        32 * cc : 32 * (cc + 1), (base + 1) * TC : (base + 33) * TC
                    ].rearrange("b (i x) -> b i x", x=2 * TC)[:, :, 0:TC]
                    if h == 0 and cc == 0:
                        # blank block first: the state-0 cumprod (and thus
                        # the whole scan chain) depends on it
                        nc.sync.dma_start(dsto, srcb)
                        nc.sync.dma_start(dste, src)
                    else:
                        nc.sync.dma_start(dste, src)
                        nc.sync.dma_start(dsto, srcb)

            if debug:
                nc.sync.dma_start(ysk_d.ap()[:, :], ylab_sk[:])
                nc.sync.dma_start(gbd_d.ap()[:, :], gbd[:, :])

            # ---- scan phase ----
            def d1_view(k):
                return ylab_sk[:, k * TC : (k + 1) * TC]

            for k in range(NWAVE):
                if k == 0:
                    # state-0 (blank lane) cumprod, chunk 0 only: the series
                    # underflows bf16 to zero well before chunk 1, and the
                    # zero-memset ring tiles already supply zeros for chunks
                    # 1-3. Written into ring[5] group-0 rows so wavefront 0's
                    # packed d0 read sees it as "aw_{-1}".
                    slot = ring[5]
                    pbv = ylab_sk[0:32, TC : 2 * TC]  # pblank chunk 0 (r=1)
                    nc.vector.tensor_tensor_scan(
                        slot[0:32, 1 : TC + 1], pbv, pbv, 1.0, op0=MULT, op1=BYP
                    )

                slot_out = ring[k % 6]
                slot_1 = ring[(k - 1) % 6]
                slot_2 = ring[(k - 2) % 6]
                # d0t tile: DVE pre-writes the full skip term wm =
                # mv2wave[:,k] * STORED(s-2) (incl. boundary col 0, whose
                # slot_2 col 0 is the 2-cycle-old carry copy). Off chain (2
                # cycles of slack).
                d0t = w_pool.tile([128, TC], bf, tag="d0t", name=f"d0t{k}")
                nc.vector.tensor_scalar(
                    d0t[:], slot_2[:, 0:TC], mw_sb[:, k : k + 1], None, op0=MULT
                )
                # d0 accumulates in PSUM on the PE so the only critical-path
                # step between scans is one 255-col diag matmul:
                #   d0s  = I @ wm                       (early: wm ready)
                #   d0s[0]   += sh32 @ slot_1[TC]       (own-carry; initial
                #   d0s[0]   += shc1 @ slot_2[TC]        and a1-carry folded
                #                                        into col 0)
                #   d0s[1:]  += diag(c1) @ slot_1[1:TC] (on-path, after
                #                                        scan_{k-1})
                # The scan then runs with initial=0.
                d0s = pstr_pool.tile([128, TC], f32, tag="pstr", name=f"d0s{k}")
                nc.tensor.matmul(d0s[:], id_sb[:], d0t[:], start=True, stop=False)
                shp = pst_pool.tile([128, 8], f32, tag="pst", name=f"shp{k}")
                nc.tensor.matmul(
                    shp[:, 1:2], sh_sb[:], slot_1[:, TC : TC + 1],
                    start=True, stop=True,
                )
                if k == 0:
                    # ring[4] col 0 carries the state-0 t=-1 boundary (1.0):
                    # cover col 0 with the diag matmul directly.
                    nc.tensor.matmul(
                        d0s[:, 0:1], sh_sb[:], slot_1[:, TC : TC + 1],
                        start=False, stop=False,
                    )
                    nc.tensor.matmul(
                        d0s[:, 0:TC], dc_sb[:, 0:128], slot_1[:, 0:TC],
                        start=False, stop=True,
                    )
                else:
                    nc.tensor.matmul(
                        d0s[:, 0:1], sh_sb[:], slot_1[:, TC : TC + 1],
                        start=False, stop=False,
                    )
                    nc.tensor.matmul(
                        d0s[:, 0:1],
                        shc_sb[:, (k % 2) * 128 : (k % 2) * 128 + 128],
                        slot_2[:, TC : TC + 1],
                        start=False, stop=False,
                    )
                    nc.tensor.matmul(
                        d0s[:, 1:TC],
                        dc_sb[:, (k % 2) * 128 : (k % 2) * 128 + 128],
                        slot_1[:, 1:TC],
                        start=False, stop=True,
                    )
                nc.vector.tensor_tensor_scan(
                    slot_out[:, 1 : TC + 1],
                    d0s[:, 0:TC],
                    d1_view(k),
                    0.0,
                    op0=ADD,
                    op1=MULT,
                )
                # boundary col 0 of THIS slot (consumed by the wm op of
                # wavefront k+2, which has ~2 cycles of slack). Emitted after
                # the scan so the scheduler doesn't gate the scan behind it.
                nc.scalar.copy(ring[k % 6][:, 0:1], shp[:, 1:2])
                if debug:
                    nc.sync.dma_start(aw_d.ap()[k], slot_out[:])
                    if k < 6:
                        nc.sync.dma_start(d0_d.ap()[k], d0t[:])
                        nc.sync.dma_start(wm_d.ap()[k], wm[:])
                        nc.sync.dma_start(w4_d.ap()[k], w4[:])
                        shcp = w_pool.tile([128, 1], f32, tag="shcp", name=f"shcp{k}")
                        nc.scalar.copy(shcp[:], shp[:, 0:1])
                        nc.sync.dma_start(sh_dump.ap()[k], shcp[:])

            # ---- final ----
            # STORED[127] from wavefront 129 (ring[4]), STORED[128] from 130
            # (ring[0]); both group 3, last col.
            xa = ring[129 % 6][96:128, TC : TC + 1]
            xb = ring[130 % 6][96:128, TC : TC + 1]
            xt = fin_pool.tile([128, 1], f32, tag="x")
            nc.vector.tensor_tensor(xt[96:128, :], xa, xb, op=ADD)
            lnx = fin_pool.tile([128, 1], f32, tag="lnx")
            nc.scalar.activation(
                lnx[96:128, :],
                xt[96:128, :],
                mybir.ActivationFunctionType.Ln,
                scale=float(2.0**-64),
            )
            res = fin_pool.tile([128, 1], f32, tag="res")
            nc.vector.tensor_scalar(res[96:128, :], lnx[96:128, :], -1.0, K_FIN, MULT, ADD)
            nc.sync.dma_start(out_d.ap()[:, :], res[96:128, :])

    nc.compile()
    return nc


def _host_inputs(y_true, y_pred):
    """Per-core input maps."""
    bf16 = ml_dtypes.bfloat16
    # shared constants
    # family tt: moving col j -> t = 8j + tt, source partition j
    dsched = np.zeros((128, 8 * 128), dtype=bf16)
    for tt in range(8):
        for j in range(128):
            dsched[j, tt * 128 + j] = bf16(C_SCHED[8 * j + tt])
    v2tab = np.zeros((128, 3), dtype=np.float32)
    for c in range(4):
        for j in range(2):
            v2tab[32 * c : 32 * (c + 1), j] = V2 if (c % 2) == j else 1.0
    v2tab[:, 2] = EPS
    shift32 = np.zeros((128, 128), dtype=bf16)
    for p in range(96):
        shift32[p, p + 32] = bf16(1.0)
    # shift32 pre-scaled by c1 (k-parity variants j=0,1): out[p] = c1_j[p] *
    # src[p-32] -> stationary[q, j*128 + p] = v2tab[p, j] at p = q+32.
    shiftc1 = np.zeros((128, 2 * 128), dtype=bf16)
    for q in range(96):
        for j in range(2):
            shiftc1[q, j * 128 + q + 32] = bf16(v2tab[q + 32, j])
    ident = np.zeros((128, 128), dtype=bf16)
    for q in range(128):
        ident[q, q] = bf16(1.0)
    # diag(c1) per k-parity: diagc1[q, j*128 + q] = v2tab[q, j]
    diagc1 = np.zeros((128, 2 * 128), dtype=bf16)
    for q in range(128):
        for j in range(2):
            diagc1[q, j * 128 + q] = bf16(v2tab[q, j])

    in_maps = []
    for i in range(NCORES):
        sl = slice(i * BPC, (i + 1) * BPC)
        lab = np.asarray(y_true[sl], dtype=np.int64)  # [32, 64]
        # rows 0..63 = label lanes, rows 64..79 = blank copies
        emat = np.zeros((C, BPC * NE), dtype=bf16)
        for b in range(BPC):
            for l in range(L):
                emat[lab[b, l], b * NE + l] = bf16(1.0)
            emat[BLANK, b * NE + 64 : b * NE + 80] = bf16(1.0)
        mv2wave = np.zeros((128, NWAVE), dtype=np.float32)
        for c in range(4):
            for k in range(NWAVE):
                s = k + 1 - c
                if 3 <= s <= NR and s % 2 == 1:
                    l = (s - 1) // 2
                    mv2wave[32 * c : 32 * (c + 1), k] = (
                        lab[:, l] != lab[:, l - 1]
                    ).astype(np.float32) * V2
        in_maps.append(
            {
                "y_pred": np.ascontiguousarray(
                    (np.asarray(y_pred[sl], np.float32) + np.float32(EPS)).astype(bf16)
                ),
                "emat": emat,
                "dsched": dsched,
                "mv2wave": mv2wave,
                "shift32": shift32,
                "shiftc1": shiftc1,
                "ident": ident,
                "diagc1": diagc1,
            }
        )
    return in_maps


def kernel(y_true, y_pred):
    global _PROGRAM
    from concourse.bass_utils import run_bass_kernel_spmd

    y_true = np.asarray(y_true)
    y_pred = np.asarray(y_pred, dtype=np.float32)
    if _PROGRAM is None:
        _PROGRAM = _build_program()
    in_maps = _host_inputs(y_true, y_pred)
    r = run_bass_kernel_spmd(_PROGRAM, in_maps, list(range(NCORES)))
    out = np.concatenate([r.results[i]["out"] for i in range(NCORES)], axis=0)
    return out.astype(np.float32)

